# revision 15
# baseline (speedup 1.0000x reference)
"""BilinearPooling Trainium2 kernel — hand-scheduled raw Bacc version.

    out[b,:] = audio[b,:] * s_b / max(|s_b| * ||audio_b||, eps),
    s_b = rowsum(visual[b,:])

Data parallel across 8 NeuronCores (1024 rows/core). Each engine's
instruction stream is written explicitly with manual semaphores, which
drops Tile's kernel epilogue (drain + double all-engine barrier ~9 us)
and start barrier. TRN2 engines are deep pipelines with no operand
interlock, so EVERY data dependency — including producer/consumer on
the same engine — is ordered through a semaphore: each engine has a
chain semaphore that every instruction bumps on retire, and consumers
wait for the producer's count (cross-engine edges use dedicated sems).

DMA plan: packet-level profiling shows each of the 16 SDMA engines
runs at ~25.5-27 GB/s — the aggregate ~410 GB/s is the per-engine
hardware ceiling, reached only when both HWDGE rings (SP + ACT) stay
fed with >=1 MiB transfers (one ring ~358; 0.5 MiB transfers ~320;
SWDGE as a third queue makes it worse). Loads move 2 MiB "pairs" of
row-tiles: DRAM rows 256j..256j+255 viewed as "(p k) d" so partition p
holds rows 2p, 2p+1 contiguously -> one flat 2 MiB range per transfer;
the last pair is loaded as two 1 MiB halves so the drain chain stays
short. Stores are contiguous 2 MiB per pair (pairs 0-2) plus two 1 MiB
tile stores for the last pair. Ring balance 12.6 MB each: audio loads
+ pair-1 store + last-tile stores on SP; visual loads + pair-0/2
stores on ACT — both rings stay busy to the very end (measured: zero
stream gaps >150 ns, rings finish within 0.3 us). All pair buffers are
distinct (16 MiB input set resident in SBUF): no write-after-read
hazards, all loads queue immediately.

Row-tile t = 2j+k (pair j, sub-tile k), column halves h of 1024 cols.
DVE: rowsum halves, tiny scale chain, odd-half multiply. ACT: square+
accumulate halves (into two alternating write-only scratches), sqrt,
even-half multiply. Fast-core profile: ~6.2 us runtime preamble +
~61 us gap-free DMA at ~410 GB/s + ~8 us runtime epilogue ~= 77 us;
externally HBM-contended cores stretch the DMA phase.
"""

from contextlib import ExitStack

import numpy as np

import concourse.bass as bass
from concourse import mybir
from concourse.bacc import Bacc
from concourse.bass_utils import run_bass_kernel_spmd

B, D = 8192, 2048
N_CORES = 8
ROWS = B // N_CORES          # 1024 rows per core
P = 128
N_TILES = ROWS // P          # 8 row-tiles
N_PAIRS = N_TILES // 2       # 4 2-MiB load pairs
D2 = D // 2
EPS = 1e-12
FP32 = mybir.dt.float32
AF = mybir.ActivationFunctionType


class ChainSync:
    """Orders data deps through one per-engine chain semaphore.

    Every producing instruction calls produce(inst, key); consumers call
    wait(engine, key) which emits a wait_ge for the producer's count.
    Engine bodies are traced in a fixed order, so forward references
    (ACT waiting on a DVE mark) need a counting pre-pass: build once
    with preset=None (waits no-op, counts recorded), then rebuild with
    the recorded marks.
    """

    def __init__(self, sem, preset=None):
        self.sem = sem
        self.count = 0
        self.marks = {}
        self.preset = preset

    def produce(self, inst, key=None):
        if inst is not None:
            inst.then_inc(self.sem, 1)
        self.count += 1
        if key is not None:
            self.marks[key] = self.count
        return inst

    def wait(self, engine, key):
        if self.preset is not None:
            engine.wait_ge(self.sem, self.preset[key])


def build_bass(_marks=None):
    nc = Bacc()
    audio = nc.declare_dram_parameter("audio", [ROWS, D], FP32, isOutput=False)
    visual = nc.declare_dram_parameter("visual", [ROWS, D], FP32, isOutput=False)
    out = nc.declare_dram_parameter("out", [ROWS, D], FP32, isOutput=True)

    # DRAM views: pair j covers rows 256j..256j+255; "(p k) d" puts rows
    # 2p, 2p+1 on partition p -> the whole pair is one contiguous 2 MiB
    # DRAM range and 16 KiB contiguous per partition.
    a_pairs_dram = [
        audio[256 * j : 256 * (j + 1), :].rearrange("(p k) d -> p (k d)", k=2)
        for j in range(N_PAIRS)
    ]
    v_pairs_dram = [
        visual[256 * j : 256 * (j + 1), :].rearrange("(p k) d -> p (k d)", k=2)
        for j in range(N_PAIRS)
    ]
    o_pairs_dram = [
        out[256 * j : 256 * (j + 1), :].rearrange("(p k) d -> p (k d)", k=2)
        for j in range(N_PAIRS)
    ]
    o_tiles_dram = [
        out[256 * j : 256 * (j + 1), :].rearrange("(p k) d -> p (k d)", k=2)[
            :, k * D : (k + 1) * D
        ]
        for j in range(N_PAIRS)
        for k in range(2)
    ]

    with ExitStack() as ctx:
        a_bufs = [
            ctx.enter_context(nc.sbuf_tensor(f"a_buf{j}", [P, 2 * D], FP32))
            for j in range(N_PAIRS)
        ]
        v_bufs = [
            ctx.enter_context(nc.sbuf_tensor(f"v_buf{j}", [P, 2 * D], FP32))
            for j in range(N_PAIRS)
        ]
        scr = [
            ctx.enter_context(nc.sbuf_tensor(f"scr{h}", [P, D2], FP32))
            for h in range(2)
        ]
        zero = ctx.enter_context(nc.sbuf_tensor("zero", [P, 1], FP32))
        s2 = ctx.enter_context(nc.sbuf_tensor("s2", [P, 2 * N_TILES], FP32))
        q2 = ctx.enter_context(nc.sbuf_tensor("q2", [P, 2 * N_TILES], FP32))
        s_ = ctx.enter_context(nc.sbuf_tensor("s_", [P, N_TILES], FP32))
        pp = ctx.enter_context(nc.sbuf_tensor("pp", [P, N_TILES], FP32))
        n2 = ctx.enter_context(nc.sbuf_tensor("n2", [P, N_TILES], FP32))
        rr = ctx.enter_context(nc.sbuf_tensor("rr", [P, N_TILES], FP32))
        sc = ctx.enter_context(nc.sbuf_tensor("sc", [P, N_TILES], FP32))

        # One sem per load transfer: transfers on a ring may complete out
        # of order, so a shared counting sem would be unsound. The last
        # pair is loaded as two 1 MiB halves (extra sem each) so the final
        # tiles' compute starts earlier and the drain chain is short.
        A_ = [ctx.enter_context(nc.semaphore(f"A{j}")) for j in range(N_PAIRS + 1)]
        V_ = [ctx.enter_context(nc.semaphore(f"V{j}")) for j in range(N_PAIRS + 1)]
        ST = ctx.enter_context(nc.semaphore("ST"))
        DVC = ctx.enter_context(nc.semaphore("DVC"))
        ACC = ctx.enter_context(nc.semaphore("ACC"))

        dv = ChainSync(DVC, preset=None if _marks is None else _marks[0])
        ac = ChainSync(ACC, preset=None if _marks is None else _marks[1])

        # sub-views: row-tile t=2j+k lives in pair buffer j, cols [kD,(k+1)D)
        def a_tile(t):
            return a_bufs[t // 2][:, (t % 2) * D : (t % 2 + 1) * D]

        def v_tile(t):
            return v_bufs[t // 2][:, (t % 2) * D : (t % 2 + 1) * D]

        block = ctx.enter_context(nc.Block())

        @block.sync
        def _(sp):
            for j in range(N_PAIRS - 1):
                sp.dma_start(out=a_bufs[j][:, :], in_=a_pairs_dram[j]).then_inc(A_[j], 16)
            jl = N_PAIRS - 1
            for k in range(2):
                sp.dma_start(
                    out=a_bufs[jl][:, k * D : (k + 1) * D],
                    in_=a_pairs_dram[jl][:, k * D : (k + 1) * D],
                ).then_inc(A_[jl + k], 16)
            for t in (2, 3):
                dv.wait(sp, ("mh1", t))
                ac.wait(sp, ("m0", t))
            sp.dma_start(out=o_pairs_dram[1], in_=a_bufs[1][:, :]).then_inc(ST, 16)
            for t in (6, 7):
                dv.wait(sp, ("mh1", t))
                ac.wait(sp, ("m0", t))
                sp.dma_start(out=o_tiles_dram[t], in_=a_tile(t)).then_inc(ST, 16)
            # all stores landed -> output durable in HBM (5 stores total)
            sp.wait_ge(ST, 16 * 5)

        @block.scalar
        def _(act):
            for j in range(N_PAIRS - 1):
                act.dma_start(out=v_bufs[j][:, :], in_=v_pairs_dram[j]).then_inc(V_[j], 16)
            jl = N_PAIRS - 1
            for k in range(2):
                act.dma_start(
                    out=v_bufs[jl][:, k * D : (k + 1) * D],
                    in_=v_pairs_dram[jl][:, k * D : (k + 1) * D],
                ).then_inc(V_[jl + k], 16)
            ac.produce(nc.scalar.memzero(zero[:, :]), "z")
            ac.wait(act, "z")

            def mul_only(t):
                dv.wait(act, ("sc", t))
                ac.produce(
                    nc.scalar.activation(
                        out=a_tile(t)[:, 0:D2],
                        in_=a_tile(t)[:, 0:D2],
                        func=AF.Copy,
                        scale=sc[:, t : t + 1],
                    ),
                    ("m0", t),
                )

            def store_pair(j):
                for t in (2 * j, 2 * j + 1):
                    dv.wait(act, ("mh1", t))
                    ac.wait(act, ("m0", t))
                act.dma_start(out=o_pairs_dram[j], in_=a_bufs[j][:, :]).then_inc(ST, 16)

            for t in range(N_TILES):
                act.wait_ge(A_[t // 2 if t < 2 * (N_PAIRS - 1) else N_PAIRS - 1 + t % 2], 16)
                for h in range(2):
                    # WAW on scr[h] with the same-h square two ops back;
                    # by then it has long retired, so this wait is free.
                    if t >= 1:
                        ac.wait(act, ("sq", 2 * (t - 1) + h))
                    sq = nc.scalar.activation(
                        out=scr[h][:, :],
                        in_=a_tile(t)[:, h * D2 : (h + 1) * D2],
                        func=AF.Square,
                        bias=zero[:, :],
                        accum_out=q2[:, 2 * t + h : 2 * t + h + 1],
                    )
                    ac.produce(sq, ("sq", 2 * t + h))
                if t >= 1:
                    mul_only(t - 1)
                if t == 2:
                    store_pair(0)
                elif t == 6:
                    store_pair(2)
                dv.wait(act, ("n2", t))
                ac.produce(
                    nc.scalar.activation(
                        out=rr[:, t : t + 1],
                        in_=n2[:, t : t + 1],
                        func=AF.Sqrt,
                        bias=zero[:, :],
                    ),
                    ("sqrt", t),
                )
            mul_only(N_TILES - 1)

        @block.vector
        def _(dve):
            def chain2_mul(t):
                ac.wait(dve, ("sqrt", t))
                dv.produce(
                    nc.vector.tensor_scalar_max(
                        out=rr[:, t : t + 1], in0=rr[:, t : t + 1], scalar1=EPS
                    ),
                    ("mx", t),
                )
                dv.wait(dve, ("mx", t))
                dv.produce(
                    nc.vector.reciprocal(out=rr[:, t : t + 1], in_=rr[:, t : t + 1]),
                    ("rc", t),
                )
                dv.wait(dve, ("rc", t))
                dv.produce(
                    nc.vector.tensor_mul(
                        out=sc[:, t : t + 1],
                        in0=s_[:, t : t + 1],
                        in1=rr[:, t : t + 1],
                    ),
                    ("sc", t),
                )
                dv.wait(dve, ("sc", t))
                dv.produce(
                    nc.vector.tensor_scalar_mul(
                        out=a_tile(t)[:, D2:D],
                        in0=a_tile(t)[:, D2:D],
                        scalar1=sc[:, t : t + 1],
                    ),
                    ("mh1", t),
                )

            for t in range(N_TILES):
                dve.wait_ge(V_[t // 2 if t < 2 * (N_PAIRS - 1) else N_PAIRS - 1 + t % 2], 16)
                for h in range(2):
                    dv.produce(
                        nc.vector.reduce_sum(
                            out=s2[:, 2 * t + h : 2 * t + h + 1],
                            in_=v_tile(t)[:, h * D2 : (h + 1) * D2],
                            axis=mybir.AxisListType.X,
                        ),
                        ("s2", 2 * t + h),
                    )
                if t >= 1:
                    chain2_mul(t - 1)
                dv.wait(dve, ("s2", 2 * t + 1))
                ac.wait(dve, ("sq", 2 * t + 1))
                dv.produce(
                    nc.vector.tensor_add(
                        out=s_[:, t : t + 1],
                        in0=s2[:, 2 * t : 2 * t + 1],
                        in1=s2[:, 2 * t + 1 : 2 * t + 2],
                    ),
                    ("s_", t),
                )
                dv.produce(
                    nc.vector.tensor_add(
                        out=n2[:, t : t + 1],
                        in0=q2[:, 2 * t : 2 * t + 1],
                        in1=q2[:, 2 * t + 1 : 2 * t + 2],
                    ),
                    ("qs", t),
                )
                dv.wait(dve, ("s_", t))
                dv.produce(
                    nc.vector.tensor_mul(
                        out=pp[:, t : t + 1], in0=s_[:, t : t + 1], in1=s_[:, t : t + 1]
                    ),
                    ("pp", t),
                )
                dv.wait(dve, ("pp", t))
                dv.produce(
                    nc.vector.tensor_mul(
                        out=n2[:, t : t + 1], in0=n2[:, t : t + 1], in1=pp[:, t : t + 1]
                    ),
                    ("n2", t),
                )
            chain2_mul(N_TILES - 1)

    if _marks is None:
        # counting pass done: rebuild with the mark tables so waits on
        # forward references (ACT waiting on DVE marks) can be emitted.
        return build_bass(_marks=(dv.marks, ac.marks))
    nc.finalize()
    return nc


_NC = None


def _get_nc():
    global _NC
    if _NC is None:
        _NC = build_bass()
    return _NC


def kernel(audio: np.ndarray, visual: np.ndarray) -> np.ndarray:
    audio = np.ascontiguousarray(audio, dtype=np.float32)
    visual = np.ascontiguousarray(visual, dtype=np.float32)
    nc = _get_nc()
    in_maps = [
        {
            "audio": audio[i * ROWS : (i + 1) * ROWS],
            "visual": visual[i * ROWS : (i + 1) * ROWS],
        }
        for i in range(N_CORES)
    ]
    res = run_bass_kernel_spmd(nc, in_maps, core_ids=list(range(N_CORES)))
    return np.concatenate([r["out"] for r in res.results], axis=0)


# revision 17
# speedup vs baseline: 1.0267x; 1.0267x over previous
"""BilinearPooling Trainium2 kernel — hand-scheduled raw Bacc version.

    out[b,:] = audio[b,:] * s_b / max(|s_b| * ||audio_b||, eps),
    s_b = rowsum(visual[b,:])

Data parallel across 8 NeuronCores (1024 rows/core). Each engine's
instruction stream is written explicitly with manual semaphores, which
drops Tile's kernel epilogue (drain + double all-engine barrier ~9 us)
and start barrier. TRN2 engines are deep pipelines with no operand
interlock, so EVERY data dependency — including producer/consumer on
the same engine — is ordered through a semaphore: each engine has a
chain semaphore that every instruction bumps on retire, and consumers
wait for the producer's count (cross-engine edges use dedicated sems).

DMA plan: packet-level profiling shows each of the 16 SDMA engines
caps at ~25.5-27 GB/s, so the aggregate ~410 GB/s ceiling is reached
only when both HWDGE rings (SP + ACT) stay fed with >=1 MiB transfers
(one ring ~358 GB/s; 0.5 MiB transfers ~320; SWDGE as a third queue is
worse). Loads move 2 MiB "pairs" of row-tiles: DRAM rows 256j..256j+255
viewed as "(p k) d" so partition p holds rows 2p, 2p+1 contiguously ->
one flat 2 MiB range per transfer; the last pair is loaded as two
1 MiB halves so the drain chain stays short. Stores are contiguous
2 MiB per pair (pairs 0-2) plus two 1 MiB tile stores for the last
pair. Ring balance 12.6 MB each: audio loads + pair-1 store +
last-tile stores on SP; visual loads + pair-0/2 stores on ACT — both
rings stay busy to the very end (measured: zero stream gaps >150 ns).
All pair buffers are distinct (16 MiB input set resident in SBUF): no
write-after-read hazards, all loads queue immediately. Bass's
unconditional entry-block const memsets + barrier are stripped before
finalize (unused here), shaving ~0.6 us of preamble.

Row-tile t = 2j+k (pair j, sub-tile k), column halves h of 1024 cols.
DVE: rowsum halves, tiny scale chain, odd-half multiply. ACT: square+
accumulate halves (into two alternating write-only scratches), sqrt,
even-half multiply. Fast-core profile: ~5.6 us runtime preamble +
~61 us gap-free DMA at ~410 GB/s + ~8 us runtime epilogue ~= 76 us;
externally HBM-contended cores stretch the DMA phase.
"""

from contextlib import ExitStack

import numpy as np

import concourse.bass as bass
from concourse import mybir
from concourse.bacc import Bacc
from concourse.bass_utils import run_bass_kernel_spmd

B, D = 8192, 2048
N_CORES = 8
ROWS = B // N_CORES          # 1024 rows per core
P = 128
N_TILES = ROWS // P          # 8 row-tiles
N_PAIRS = N_TILES // 2       # 4 2-MiB load pairs
D2 = D // 2
EPS = 1e-12
FP32 = mybir.dt.float32
AF = mybir.ActivationFunctionType


class ChainSync:
    """Orders data deps through one per-engine chain semaphore.

    Every producing instruction calls produce(inst, key); consumers call
    wait(engine, key) which emits a wait_ge for the producer's count.
    Engine bodies are traced in a fixed order, so forward references
    (ACT waiting on a DVE mark) need a counting pre-pass: build once
    with preset=None (waits no-op, counts recorded), then rebuild with
    the recorded marks.
    """

    def __init__(self, sem, preset=None):
        self.sem = sem
        self.count = 0
        self.marks = {}
        self.preset = preset

    def produce(self, inst, key=None):
        if inst is not None:
            inst.then_inc(self.sem, 1)
        self.count += 1
        if key is not None:
            self.marks[key] = self.count
        return inst

    def wait(self, engine, key):
        if self.preset is not None:
            engine.wait_ge(self.sem, self.preset[key])


def build_bass(_marks=None):
    nc = Bacc()
    audio = nc.declare_dram_parameter("audio", [ROWS, D], FP32, isOutput=False)
    visual = nc.declare_dram_parameter("visual", [ROWS, D], FP32, isOutput=False)
    out = nc.declare_dram_parameter("out", [ROWS, D], FP32, isOutput=True)

    # DRAM views: pair j covers rows 256j..256j+255; "(p k) d" puts rows
    # 2p, 2p+1 on partition p -> the whole pair is one contiguous 2 MiB
    # DRAM range and 16 KiB contiguous per partition.
    a_pairs_dram = [
        audio[256 * j : 256 * (j + 1), :].rearrange("(p k) d -> p (k d)", k=2)
        for j in range(N_PAIRS)
    ]
    v_pairs_dram = [
        visual[256 * j : 256 * (j + 1), :].rearrange("(p k) d -> p (k d)", k=2)
        for j in range(N_PAIRS)
    ]
    o_pairs_dram = [
        out[256 * j : 256 * (j + 1), :].rearrange("(p k) d -> p (k d)", k=2)
        for j in range(N_PAIRS)
    ]
    o_tiles_dram = [
        out[256 * j : 256 * (j + 1), :].rearrange("(p k) d -> p (k d)", k=2)[
            :, k * D : (k + 1) * D
        ]
        for j in range(N_PAIRS)
        for k in range(2)
    ]

    with ExitStack() as ctx:
        a_bufs = [
            ctx.enter_context(nc.sbuf_tensor(f"a_buf{j}", [P, 2 * D], FP32))
            for j in range(N_PAIRS)
        ]
        v_bufs = [
            ctx.enter_context(nc.sbuf_tensor(f"v_buf{j}", [P, 2 * D], FP32))
            for j in range(N_PAIRS)
        ]
        scr = [
            ctx.enter_context(nc.sbuf_tensor(f"scr{h}", [P, D2], FP32))
            for h in range(2)
        ]
        zero = ctx.enter_context(nc.sbuf_tensor("zero", [P, 1], FP32))
        s2 = ctx.enter_context(nc.sbuf_tensor("s2", [P, 2 * N_TILES], FP32))
        q2 = ctx.enter_context(nc.sbuf_tensor("q2", [P, 2 * N_TILES], FP32))
        s_ = ctx.enter_context(nc.sbuf_tensor("s_", [P, N_TILES], FP32))
        pp = ctx.enter_context(nc.sbuf_tensor("pp", [P, N_TILES], FP32))
        n2 = ctx.enter_context(nc.sbuf_tensor("n2", [P, N_TILES], FP32))
        rr = ctx.enter_context(nc.sbuf_tensor("rr", [P, N_TILES], FP32))
        sc = ctx.enter_context(nc.sbuf_tensor("sc", [P, N_TILES], FP32))

        # One sem per load transfer: transfers on a ring may complete out
        # of order, so a shared counting sem would be unsound. The last
        # pair is loaded as two 1 MiB halves (extra sem each) so the final
        # tiles' compute starts earlier and the drain chain is short.
        A_ = [ctx.enter_context(nc.semaphore(f"A{j}")) for j in range(N_PAIRS + 1)]
        V_ = [ctx.enter_context(nc.semaphore(f"V{j}")) for j in range(N_PAIRS + 1)]
        ST = ctx.enter_context(nc.semaphore("ST"))
        DVC = ctx.enter_context(nc.semaphore("DVC"))
        ACC = ctx.enter_context(nc.semaphore("ACC"))

        dv = ChainSync(DVC, preset=None if _marks is None else _marks[0])
        ac = ChainSync(ACC, preset=None if _marks is None else _marks[1])

        # sub-views: row-tile t=2j+k lives in pair buffer j, cols [kD,(k+1)D)
        def a_tile(t):
            return a_bufs[t // 2][:, (t % 2) * D : (t % 2 + 1) * D]

        def v_tile(t):
            return v_bufs[t // 2][:, (t % 2) * D : (t % 2 + 1) * D]

        block = ctx.enter_context(nc.Block())

        @block.sync
        def _(sp):
            for j in range(N_PAIRS - 1):
                sp.dma_start(out=a_bufs[j][:, :], in_=a_pairs_dram[j]).then_inc(A_[j], 16)
            jl = N_PAIRS - 1
            for k in range(2):
                sp.dma_start(
                    out=a_bufs[jl][:, k * D : (k + 1) * D],
                    in_=a_pairs_dram[jl][:, k * D : (k + 1) * D],
                ).then_inc(A_[jl + k], 16)
            for t in (2, 3):
                dv.wait(sp, ("mh1", t))
                ac.wait(sp, ("m0", t))
            sp.dma_start(out=o_pairs_dram[1], in_=a_bufs[1][:, :]).then_inc(ST, 16)
            for t in (6, 7):
                dv.wait(sp, ("mh1", t))
                ac.wait(sp, ("m0", t))
                sp.dma_start(out=o_tiles_dram[t], in_=a_tile(t)).then_inc(ST, 16)
            # all stores landed -> output durable in HBM (5 stores total)
            sp.wait_ge(ST, 16 * 5)

        @block.scalar
        def _(act):
            for j in range(N_PAIRS - 1):
                act.dma_start(out=v_bufs[j][:, :], in_=v_pairs_dram[j]).then_inc(V_[j], 16)
            jl = N_PAIRS - 1
            for k in range(2):
                act.dma_start(
                    out=v_bufs[jl][:, k * D : (k + 1) * D],
                    in_=v_pairs_dram[jl][:, k * D : (k + 1) * D],
                ).then_inc(V_[jl + k], 16)
            ac.produce(nc.scalar.memzero(zero[:, :]), "z")
            ac.wait(act, "z")

            def mul_only(t):
                dv.wait(act, ("sc", t))
                ac.produce(
                    nc.scalar.activation(
                        out=a_tile(t)[:, 0:D2],
                        in_=a_tile(t)[:, 0:D2],
                        func=AF.Copy,
                        scale=sc[:, t : t + 1],
                    ),
                    ("m0", t),
                )

            def store_pair(j):
                for t in (2 * j, 2 * j + 1):
                    dv.wait(act, ("mh1", t))
                    ac.wait(act, ("m0", t))
                act.dma_start(out=o_pairs_dram[j], in_=a_bufs[j][:, :]).then_inc(ST, 16)

            for t in range(N_TILES):
                act.wait_ge(A_[t // 2 if t < 2 * (N_PAIRS - 1) else N_PAIRS - 1 + t % 2], 16)
                for h in range(2):
                    # WAW on scr[h] with the same-h square two ops back;
                    # by then it has long retired, so this wait is free.
                    if t >= 1:
                        ac.wait(act, ("sq", 2 * (t - 1) + h))
                    sq = nc.scalar.activation(
                        out=scr[h][:, :],
                        in_=a_tile(t)[:, h * D2 : (h + 1) * D2],
                        func=AF.Square,
                        bias=zero[:, :],
                        accum_out=q2[:, 2 * t + h : 2 * t + h + 1],
                    )
                    ac.produce(sq, ("sq", 2 * t + h))
                if t >= 1:
                    mul_only(t - 1)
                if t == 2:
                    store_pair(0)
                elif t == 6:
                    store_pair(2)
                dv.wait(act, ("n2", t))
                ac.produce(
                    nc.scalar.activation(
                        out=rr[:, t : t + 1],
                        in_=n2[:, t : t + 1],
                        func=AF.Sqrt,
                        bias=zero[:, :],
                    ),
                    ("sqrt", t),
                )
            mul_only(N_TILES - 1)

        @block.vector
        def _(dve):
            def chain2_mul(t):
                ac.wait(dve, ("sqrt", t))
                dv.produce(
                    nc.vector.tensor_scalar_max(
                        out=rr[:, t : t + 1], in0=rr[:, t : t + 1], scalar1=EPS
                    ),
                    ("mx", t),
                )
                dv.wait(dve, ("mx", t))
                dv.produce(
                    nc.vector.reciprocal(out=rr[:, t : t + 1], in_=rr[:, t : t + 1]),
                    ("rc", t),
                )
                dv.wait(dve, ("rc", t))
                dv.produce(
                    nc.vector.tensor_mul(
                        out=sc[:, t : t + 1],
                        in0=s_[:, t : t + 1],
                        in1=rr[:, t : t + 1],
                    ),
                    ("sc", t),
                )
                dv.wait(dve, ("sc", t))
                dv.produce(
                    nc.vector.tensor_scalar_mul(
                        out=a_tile(t)[:, D2:D],
                        in0=a_tile(t)[:, D2:D],
                        scalar1=sc[:, t : t + 1],
                    ),
                    ("mh1", t),
                )

            for t in range(N_TILES):
                dve.wait_ge(V_[t // 2 if t < 2 * (N_PAIRS - 1) else N_PAIRS - 1 + t % 2], 16)
                for h in range(2):
                    dv.produce(
                        nc.vector.reduce_sum(
                            out=s2[:, 2 * t + h : 2 * t + h + 1],
                            in_=v_tile(t)[:, h * D2 : (h + 1) * D2],
                            axis=mybir.AxisListType.X,
                        ),
                        ("s2", 2 * t + h),
                    )
                if t >= 1:
                    chain2_mul(t - 1)
                dv.wait(dve, ("s2", 2 * t + 1))
                ac.wait(dve, ("sq", 2 * t + 1))
                dv.produce(
                    nc.vector.tensor_add(
                        out=s_[:, t : t + 1],
                        in0=s2[:, 2 * t : 2 * t + 1],
                        in1=s2[:, 2 * t + 1 : 2 * t + 2],
                    ),
                    ("s_", t),
                )
                dv.produce(
                    nc.vector.tensor_add(
                        out=n2[:, t : t + 1],
                        in0=q2[:, 2 * t : 2 * t + 1],
                        in1=q2[:, 2 * t + 1 : 2 * t + 2],
                    ),
                    ("qs", t),
                )
                dv.wait(dve, ("s_", t))
                dv.produce(
                    nc.vector.tensor_mul(
                        out=pp[:, t : t + 1], in0=s_[:, t : t + 1], in1=s_[:, t : t + 1]
                    ),
                    ("pp", t),
                )
                dv.wait(dve, ("pp", t))
                dv.produce(
                    nc.vector.tensor_mul(
                        out=n2[:, t : t + 1], in0=n2[:, t : t + 1], in1=pp[:, t : t + 1]
                    ),
                    ("n2", t),
                )
            chain2_mul(N_TILES - 1)

    if _marks is None:
        # counting pass done: rebuild with the mark tables so waits on
        # forward references (ACT waiting on DVE marks) can be emitted.
        return build_bass(_marks=(dv.marks, ac.marks))

    # Bass.__init__ unconditionally emits four const_aps memsets plus an
    # all-engine barrier into the entry block; this kernel uses neither
    # (explicit zero tile, sem-gated engine starts), so drop them from
    # the preamble critical path. The barrier's inc/dec pairs are
    # balanced, so removing the whole set leaves the barrier sems at 0
    # for the Block-exit barrier.
    main = nc.m.functions[0].blocks[0]
    drop = [
        i
        for i in main.instructions
        if (
            type(i).__name__ == "InstMemset"
            and any(
                getattr(o, "memref", "").startswith("const-") for o in (i.outs or [])
            )
        )
        or type(i).__name__ == "InstDrain"
        or i.name.startswith("barrier_")
    ]
    for i in drop:
        main.instructions.remove(i)

    nc.finalize()
    return nc


_NC = None


def _get_nc():
    global _NC
    if _NC is None:
        _NC = build_bass()
    return _NC


def kernel(audio: np.ndarray, visual: np.ndarray) -> np.ndarray:
    audio = np.ascontiguousarray(audio, dtype=np.float32)
    visual = np.ascontiguousarray(visual, dtype=np.float32)
    nc = _get_nc()
    in_maps = [
        {
            "audio": audio[i * ROWS : (i + 1) * ROWS],
            "visual": visual[i * ROWS : (i + 1) * ROWS],
        }
        for i in range(N_CORES)
    ]
    res = run_bass_kernel_spmd(nc, in_maps, core_ids=list(range(N_CORES)))
    return np.concatenate([r["out"] for r in res.results], axis=0)
